# revision 1
# baseline (speedup 1.0000x reference)
"""DirectionalGAT Trainium2 kernel (8 NeuronCores, SPMD).

Problem (hardcoded shapes): B=4, V=20000, D=10, F=32, OUT=32, mask_index=V.

    summed   = inputs.sum(axis=2)                      # [B,V,F]
    gathered = where(adj==V, 0, summed[b, adj])        # [B,V,D,F]
    X        = (1-mask) * (gathered + initial_states)  # [B,V,D,F]
    t        = (1-mask) * relu(X @ W + b)              # [B,V,D,OUT]
    a        = t @ a_kernel                            # [B,V,D,1]
    coefs    = softmax(a - 1e7*mask, axis=D)
    out      = coefs * t

Sharding: core c -> batch b=c//2, node half h=c%2 (Vs=10000 nodes/core).
Each core computes partial node sums for its half, an AllGather within
core pairs [2b, 2b+1] assembles the full per-batch summed table [V,F] in
DRAM, and an indirect (gather) DMA with CCE-accumulate adds the gathered
rows directly onto the initial_states tiles.

On-chip layout: row-major tiles [128 v, D*F]; a DVE 32x32 block-transpose
puts (d,f) on partitions in 32-blocks so a single block-diagonal weight
matmul (lhsT=W4q) computes all four 32-row v-quarters at once.  The
attention dot / mask-count / softmax-broadcast steps are small matmuls
against static selector matrices (a4q / onesD / O4rep) writing to
base-partition 32*s so four 128-v subtiles batch into one [128,*] PSUM
region for the softmax arithmetic.  Output is written block-transposed
and un-blocked on the host.

Identities used (valid because mask is exactly 0/1 and W_bias==0):
    (1-m)*relu(((1-m)*x)@W + b) == relu(((1-m)*x)@W + b)  when b==0
    exp(a - 1e7*m) == exp(a) * (m==0)                      in fp32
    masked rows have X==0 -> t==0 -> a==0 -> exp==1.0, corrected by
    subtracting nmasked (exact in fp32).
"""

import numpy as np
from contextlib import ExitStack

import concourse.bass as bass
import concourse.bacc as bacc
import concourse.mybir as mybir
import concourse.tile as tile

F32 = mybir.dt.float32
I32 = mybir.dt.int32
I16 = mybir.dt.int16
EP = 64  # gather table row padding (dma_gather elem must be a 256B multiple)

B, V, D, F, OUT = 4, 20000, 10, 32, 32
DF = D * F          # 320
P = 128
NCORES = 8
VSH = V // 2        # 10000 nodes per core
GSUB = 4            # 128-v subtiles batched per super-tile


def _sub_starts(vsh):
    """Start rows of the 128-v subtiles, padded to a multiple of GSUB subs.

    Tail subs clamp to vsh-128 (recompute overlap; duplicate subs write
    identical values to their own output rows)."""
    n = -(-vsh // P)            # ceil
    n = -(-n // GSUB) * GSUB    # pad to multiple of GSUB
    return [min(P * i, vsh - P) for i in range(n)]


def build_nc(vsh=VSH, vfull=V, num_devices=NCORES, replica_groups=None):
    """Build the Bass program for one core (SPMD-identical across cores)."""
    if replica_groups is None:
        replica_groups = [[2 * b, 2 * b + 1] for b in range(num_devices // 2)]
    subs = _sub_starts(vsh)
    nsub = len(subs)
    nsup = nsub // GSUB
    # phase-1 tiles (cover vsh, overlap tail)
    p1 = _sub_starts(vsh)[: -(-vsh // P)]
    if p1[-1] != vsh - P:
        p1.append(vsh - P)

    nc = bacc.Bacc("TRN2", num_devices=num_devices)

    nidx = GSUB * D * P  # gathered rows per super-tile
    ncol = nidx // 16

    x_d = nc.declare_dram_parameter("x", [vsh, D, F], F32, isOutput=False)
    ini_d = nc.declare_dram_parameter("ini", [vsh, D, F], F32, isOutput=False)
    msk_d = nc.declare_dram_parameter("msk", [vsh, D], F32, isOutput=False)
    idxw_d = nc.declare_dram_parameter("idxw", [nsup * P, ncol], I16, isOutput=False)
    w4q_d = nc.declare_dram_parameter("w4q", [P, P], F32, isOutput=False)
    a4q_d = nc.declare_dram_parameter("a4q", [P, 4], F32, isOutput=False)
    onesd_d = nc.declare_dram_parameter("onesd", [P, 4], F32, isOutput=False)
    o4rep_d = nc.declare_dram_parameter("o4rep", [P, P], F32, isOutput=False)
    bblk_d = nc.declare_dram_parameter("bblk", [P, 1], F32, isOutput=False)
    out_d = nc.declare_dram_parameter("out", [nsub * P, DF], F32, isOutput=True)

    with ExitStack() as ctx:
        tc = ctx.enter_context(tile.TileContext(nc))

        dram = ctx.enter_context(tc.tile_pool(name="dram", bufs=1, space="DRAM"))
        cc_in = dram.tile([vsh, EP], F32)
        # +32 zero rows: out-of-range ids (pad) gather row `vfull` -> zeros
        summed_full = dram.tile([vfull + 32, EP], F32)

        cpool = ctx.enter_context(tc.tile_pool(name="const", bufs=1))
        w4q = cpool.tile([P, P], F32, tag="w4q")
        a4q = cpool.tile([P, 4], F32, tag="a4q")
        onesd = cpool.tile([P, 4], F32, tag="onesd")
        o4rep = cpool.tile([P, P], F32, tag="o4rep")
        bblk = cpool.tile([P, 1], F32, tag="bblk")
        nc.sync.dma_start(out=w4q[:], in_=w4q_d[:])
        nc.sync.dma_start(out=a4q[:], in_=a4q_d[:])
        nc.sync.dma_start(out=onesd[:], in_=onesd_d[:])
        nc.sync.dma_start(out=o4rep[:], in_=o4rep_d[:])
        nc.sync.dma_start(out=bblk[:], in_=bblk_d[:])

        # ---- phase 1: per-node sum over D ----
        p1pool = ctx.enter_context(tc.tile_pool(name="p1", bufs=3))
        for v0 in p1:
            xt = p1pool.tile([P, DF], F32, tag="xt")
            sm = p1pool.tile([P, F], F32, tag="sm")
            nc.sync.dma_start(
                out=xt[:], in_=x_d[v0 : v0 + P].rearrange("v d f -> v (d f)")
            )
            # reduce over d (innermost after the view)
            nc.vector.tensor_reduce(
                out=sm[:],
                in_=xt[:].rearrange("p (d f) -> p f d", d=D),
                axis=mybir.AxisListType.X,
                op=mybir.AluOpType.add,
            )
            # the (unaligned) tail tile recomputes rows already covered by
            # the previous tile; write only the new rows so no two DMAs
            # target overlapping DRAM (overlap hangs the device).
            # both 32-col halves of the padded row get sm, so cc_in is
            # fully initialized for the collective.
            lo = 128 * (vsh // 128) if v0 == vsh - P and vsh % 128 else v0
            nc.sync.dma_start(out=cc_in[lo : v0 + P, 0:F], in_=sm[lo - v0 :, :])
            nc.sync.dma_start(out=cc_in[lo : v0 + P, F:EP], in_=sm[lo - v0 :, :])

        # zero the pad rows (gathers of out-of-range ids land here)
        zt = p1pool.tile([32, EP], F32, tag="zt")
        nc.vector.memset(zt[:], 0.0)
        nc.sync.dma_start(out=summed_full[vfull : vfull + 32, :], in_=zt[:])

        # ---- all-gather the summed table within core pairs ----
        nc.gpsimd.collective_compute(
            "AllGather",
            mybir.AluOpType.bypass,
            replica_groups=replica_groups,
            ins=[cc_in[:]],
            outs=[summed_full[0:vfull, :]],
        )

        # ---- phase 2 ----
        nidreg = nc.gpsimd.to_reg(nidx)  # shared gather count register
        sb = ctx.enter_context(tc.tile_pool(name="sb", bufs=2))
        ps_y = ctx.enter_context(tc.tile_pool(name="psy", bufs=2, space="PSUM"))
        ps_a = ctx.enter_context(tc.tile_pool(name="psa", bufs=2, space="PSUM"))
        ps_n = ctx.enter_context(tc.tile_pool(name="psn", bufs=1, space="PSUM"))
        ps_s = ctx.enter_context(tc.tile_pool(name="pss", bufs=2, space="PSUM"))

        for sp in range(nsup):
            v0s = subs[sp * GSUB : (sp + 1) * GSUB]

            X = sb.tile([P, GSUB * DF], F32, tag="X")
            G = sb.tile([P, GSUB * D * EP], F32, tag="G")
            ixw = sb.tile([P, ncol], I16, tag="ixw")
            Xt = sb.tile([P, GSUB * DF], F32, tag="Xt")
            tb = sb.tile([P, GSUB * DF], F32, tag="tb")
            ob = sb.tile([P, GSUB * DF], F32, tag="ob")
            mp = sb.tile([P, P], F32, tag="mp")
            mq = sb.tile([P, P], F32, tag="mq")
            zm = sb.tile([P, GSUB * D], F32, tag="zm")
            E4 = sb.tile([P, DF], F32, tag="E4")
            szm = sb.tile([P, DF], F32, tag="szm")
            sum4 = sb.tile([P, F], F32, tag="sum4")
            r4 = sb.tile([P, F], F32, tag="r4")

            AT4 = ps_a.tile([P, DF], F32, tag="AT4")
            NM4 = ps_n.tile([P, F], F32, tag="NM4")

            nc.vector.memset(AT4[:], 0.0)
            nc.vector.memset(NM4[:], 0.0)
            nc.gpsimd.memset(mp[:], 0.0)
            nc.sync.dma_start(
                out=ixw[:], in_=idxw_d[sp * P : (sp + 1) * P, :]
            )
            for s, v0 in enumerate(v0s):
                nc.sync.dma_start(
                    out=X[:, s * DF : (s + 1) * DF],
                    in_=ini_d[v0 : v0 + P].rearrange("v d f -> v (d f)"),
                )
                nc.sync.dma_start(
                    out=mp[:, 32 * s : 32 * s + D], in_=msk_d[v0 : v0 + P, :]
                )
            # one batched gather for the whole super-tile:
            # G[p, c, :] = summed_full[idx_flat[c*128 + p], :]
            nc.gpsimd.dma_gather(
                out_ap=G[:].rearrange("p (c e) -> p c e", e=EP),
                in_ap=summed_full[:],
                idxs_ap=ixw[:],
                num_idxs=nidx,
                num_idxs_reg=nidreg,
                elem_size=EP,
                single_packet=False,
            )
            # X += gathered (first F of each padded row)
            nc.vector.tensor_tensor(
                out=X[:].rearrange("p (c f) -> p c f", f=F),
                in0=X[:].rearrange("p (c f) -> p c f", f=F),
                in1=G[:].rearrange("p (c e) -> p c e", e=EP)[:, :, 0:F],
                op=mybir.AluOpType.add,
            )

            # zm = 1 - mask  (strided view of mp)
            nc.scalar.activation(
                out=zm[:].rearrange("p (s d) -> p s d", s=GSUB),
                in_=mp[:].rearrange("p (s c) -> p s c", s=GSUB)[:, :, :D],
                func=mybir.ActivationFunctionType.Copy,
                bias=1.0,
                scale=-1.0,
            )
            # X *= zm (broadcast over f)
            nc.vector.tensor_tensor(
                out=X[:].rearrange("p (s d f) -> p s d f", s=GSUB, d=D),
                in0=X[:].rearrange("p (s d f) -> p s d f", s=GSUB, d=D),
                in1=zm[:]
                .rearrange("p (s d) -> p s d", s=GSUB)
                .to_broadcast([P, GSUB, D, F]),
                op=mybir.AluOpType.mult,
            )
            # block-transposes
            nc.vector.transpose(out=Xt[:], in_=X[:])
            nc.vector.transpose(out=mq[:], in_=mp[:])

            for s in range(GSUB):
                y = ps_y.tile([P, DF], F32, tag="y")
                nc.tensor.matmul(
                    out=y[:],
                    lhsT=w4q[:],
                    rhs=Xt[:, s * DF : (s + 1) * DF],
                    start=True,
                    stop=True,
                )
                nc.scalar.activation(
                    out=tb[:, s * DF : (s + 1) * DF],
                    in_=y[:],
                    func=mybir.ActivationFunctionType.Relu,
                    bias=bblk[:],
                )
                nc.tensor.matmul(
                    out=AT4[32 * s : 32 * s + 4, :],
                    lhsT=a4q[:],
                    rhs=tb[:, s * DF : (s + 1) * DF],
                    start=True,
                    stop=True,
                    tile_position=(0, 32 * s),
                )
                nc.tensor.matmul(
                    out=NM4[32 * s : 32 * s + 4, :],
                    lhsT=onesd[:],
                    rhs=mq[:, 32 * s : 32 * s + 32],
                    start=True,
                    stop=True,
                    tile_position=(0, 32 * s),
                )

            # softmax over d, batched over the 4 subtiles
            nc.scalar.activation(
                out=E4[:], in_=AT4[:], func=mybir.ActivationFunctionType.Exp
            )
            nc.vector.tensor_reduce(
                out=sum4[:],
                in_=E4[:].rearrange("p (d j) -> p j d", d=D),
                axis=mybir.AxisListType.X,
                op=mybir.AluOpType.add,
            )
            nc.vector.tensor_tensor(
                out=sum4[:], in0=sum4[:], in1=NM4[:], op=mybir.AluOpType.subtract
            )
            nc.vector.tensor_scalar_max(out=sum4[:], in0=sum4[:], scalar1=1e-30)
            nc.vector.reciprocal(out=r4[:], in_=sum4[:])
            nc.vector.tensor_tensor(
                out=szm[:].rearrange("p (d j) -> p j d", d=D),
                in0=E4[:].rearrange("p (d j) -> p j d", d=D),
                in1=r4[:].to_broadcast([P, F, D]),
                op=mybir.AluOpType.mult,
            )

            for s in range(GSUB):
                S = ps_s.tile([P, DF], F32, tag="S")
                nc.tensor.matmul(
                    out=S[:],
                    lhsT=o4rep[32 * s : 32 * s + 4, :],
                    rhs=szm[32 * s : 32 * s + 4, :],
                    start=True,
                    stop=True,
                    tile_position=(32 * s, 0),
                )
                nc.vector.tensor_tensor(
                    out=ob[:, s * DF : (s + 1) * DF],
                    in0=tb[:, s * DF : (s + 1) * DF],
                    in1=S[:],
                    op=mybir.AluOpType.mult,
                )
                gi = sp * GSUB + s
                nc.sync.dma_start(
                    out=out_d[gi * P : (gi + 1) * P, :],
                    in_=ob[:, s * DF : (s + 1) * DF],
                )

    nc.finalize()
    return nc


# ---------------- host side ----------------


def _consts(W_kernel, W_bias, a_kernel):
    w4q = np.zeros((P, P), np.float32)
    a4q = np.zeros((P, 4), np.float32)
    o4rep = np.zeros((P, P), np.float32)
    onesd = np.zeros((P, 4), np.float32)
    bblk = np.zeros((P, 1), np.float32)
    for g in range(4):
        w4q[32 * g : 32 * g + 32, 32 * g : 32 * g + 32] = W_kernel
        a4q[32 * g : 32 * g + 32, g] = a_kernel[:, 0]
        onesd[32 * g : 32 * g + D, g] = 1.0
        bblk[32 * g : 32 * g + 32, 0] = W_bias
        for s in range(4):
            o4rep[32 * s + g, 32 * g : 32 * g + 32] = 1.0
    return w4q, a4q, onesd, o4rep, bblk


def _wrap_idx(adj_core, vsh, vfull):
    """adj [vsh, D] (pad ids already mapped to vfull) -> the dma_gather
    int16 index tensor [nsup*128, nidx//16]: flat order i = c*128 + p with
    chunk c = (subtile s, direction d); wrapped i -> [i%16, i//16]; the 16
    wrapped partitions replicated 8x across the 128-partition tile."""
    subs = np.array(_sub_starts(vsh))
    nsub = len(subs)
    nsup = nsub // GSUB
    c = np.arange(GSUB * D)
    p = np.arange(P)
    v0s = subs.reshape(nsup, GSUB)[:, c // D]  # [nsup, GSUB*D]
    vv = v0s[:, :, None] + p[None, None, :]  # [nsup, c, p]
    dd = np.broadcast_to((c % D)[None, :, None], vv.shape)
    flat = adj_core[vv, dd].reshape(nsup, GSUB * D * P)
    wrapped = flat.reshape(nsup, -1, 16).transpose(0, 2, 1)  # [nsup, 16, ncol]
    rep = np.tile(wrapped, (1, 8, 1)).reshape(nsup * P, -1)
    return np.ascontiguousarray(rep.astype(np.int16))


def _unblock(out_dev, vsh):
    """[nsub*128, DF] block-layout -> [vsh, D, F] row-major."""
    subs = _sub_starts(vsh)
    full = np.empty((vsh, D, F), np.float32)
    for i, v0 in enumerate(subs):
        blk = out_dev[i * P : (i + 1) * P].reshape(4, 32, D, 32)  # [a,o,d,j]
        full[v0 : v0 + P] = (
            blk.transpose(0, 3, 2, 1).reshape(P, D, F)  # [a,j,d,o]
        )
    return full


_NC_CACHE = {}


def kernel(
    inputs,
    initial_states,
    mask,
    W_kernel,
    W_bias,
    a_kernel,
    adj_lst,
    mask_index,
):
    from concourse.bass_utils import run_bass_kernel_spmd

    inputs = np.asarray(inputs, np.float32)
    initial_states = np.asarray(initial_states, np.float32)
    mask = np.asarray(mask, np.float32)
    adj = np.asarray(adj_lst)
    # pad ids (== mask_index) gather the zeroed pad row at V
    adj = np.where(adj == np.asarray(mask_index), V, adj).astype(np.int32)
    w4q, a4q, onesd, o4rep, bblk = _consts(
        np.asarray(W_kernel, np.float32),
        np.asarray(W_bias, np.float32),
        np.asarray(a_kernel, np.float32),
    )

    key = "full"
    if key not in _NC_CACHE:
        _NC_CACHE[key] = build_nc()
    nc = _NC_CACHE[key]

    in_maps = []
    for c in range(NCORES):
        b, h = c // 2, c % 2
        sl = slice(h * VSH, (h + 1) * VSH)
        in_maps.append(
            {
                "x": np.ascontiguousarray(inputs[b, sl]),
                "ini": np.ascontiguousarray(initial_states[b, sl]),
                "msk": np.ascontiguousarray(mask[b, sl]),
                "idxw": _wrap_idx(adj[b, sl], VSH, V),
                "w4q": w4q,
                "a4q": a4q,
                "onesd": onesd,
                "o4rep": o4rep,
                "bblk": bblk,
            }
        )

    res = run_bass_kernel_spmd(nc, in_maps, list(range(NCORES)))
    out = np.empty((B, V, D, OUT), np.float32)
    for c in range(NCORES):
        b, h = c // 2, c % 2
        out[b, h * VSH : (h + 1) * VSH] = _unblock(res.results[c]["out"], VSH)
    return out



# revision 5
# speedup vs baseline: 3.5616x; 3.5616x over previous
"""DirectionalGAT Trainium2 kernel (8 NeuronCores, SPMD), mask-packed.

Problem (hardcoded shapes): B=4, V=20000, D=10, F=32, OUT=32, mask_index=V.

    summed   = inputs.sum(axis=2)                      # [B,V,F]
    gathered = where(adj==V, 0, summed[b, adj])        # [B,V,D,F]
    X        = (1-mask) * (gathered + initial_states)  # [B,V,D,F]
    t        = (1-mask) * relu(X @ W + b)              # [B,V,D,OUT]
    a        = t @ a_kernel                            # [B,V,D,1]
    coefs    = softmax(a - 1e7*mask, axis=D)
    out      = coefs * t

Sharding: core c -> batch b=c//2, node half h=c%2 (Vs=10000 nodes/core).
Each core computes partial node sums for its half, an AllGather within
core pairs [2b, 2b+1] assembles the full per-batch summed table [V,F] in
DRAM, and an indirect (gather) DMA fetches the per-edge rows.

Mask packing: masked edges (mask==1) contribute exactly 0 to the output
and their gathered rows are never used, so the host drops them.  Nodes
are sorted per-core by unmasked degree k (descending); each 128-node
subtile processes only k_tile = max-k-in-tile slots per node.  Slot
(v, r) holds the node's r-th unmasked direction (host-packed ini/adj);
pad slots (r >= k_v) get ini=0 and gather the zero row, so X=0, t=0,
a=0, exp=1 -- corrected by subtracting the host-computed pad count from
the softmax denominator (exact in fp32).  The subtile slot count must be
identical across cores (SPMD single program), so k_sched[i] = max over
cores.  The host unpacks the block-layout output back to [B,V,D,F] with
zeros in masked slots.

On-chip layout per super-tile (4 subtiles batched): row-major tiles
[128 v, 4*k*F]; a DVE 32x32 block-transpose puts (r,f) on partitions in
32-blocks so a single block-diagonal weight matmul (lhsT=W4q) computes
all four 32-row v-quarters at once.  The attention dot / softmax
broadcast use static selector matmuls (a4q / o4rep) at PSUM partition
offsets 32*s so the four subtiles batch into one [128,*] region.
"""

import numpy as np
from contextlib import ExitStack

import concourse.bass as bass
import concourse.bacc as bacc
import concourse.mybir as mybir
import concourse.tile as tile

F32 = mybir.dt.float32
I32 = mybir.dt.int32
I16 = mybir.dt.int16
EP = 64  # gather table row padding (dma_gather elem must be a 256B multiple)

B, V, D, F, OUT = 4, 20000, 10, 32, 32
P = 128
NCORES = 8
VSH = V // 2        # 10000 nodes per core
GSUB = 4            # 128-v subtiles batched per super-tile


def _sub_starts(vsh):
    """Start rows of the 128-v subtiles, padded to a multiple of GSUB subs.

    Tail subs clamp to vsh-128 (recompute overlap; duplicate subs write
    identical values to their own output rows)."""
    n = -(-vsh // P)            # ceil
    n = -(-n // GSUB) * GSUB    # pad to multiple of GSUB
    return [min(P * i, vsh - P) for i in range(n)]


def build_nc(k_sup, vsh=VSH, vfull=V, num_devices=NCORES, replica_groups=None):
    """Build the Bass program for one core (SPMD-identical across cores).

    k_sup: per-super-tile slot count (same for all cores)."""
    if replica_groups is None:
        replica_groups = [[2 * b, 2 * b + 1] for b in range(num_devices // 2)]
    subs = _sub_starts(vsh)
    nsub = len(subs)
    nsup = nsub // GSUB
    assert len(k_sup) == nsup
    # phase-1 tiles (cover vsh, overlap tail)
    p1 = _sub_starts(vsh)[: -(-vsh // P)]
    if p1[-1] != vsh - P:
        p1.append(vsh - P)

    nc = bacc.Bacc("TRN2", num_devices=num_devices)

    x_d = nc.declare_dram_parameter("x", [vsh, D, F], F32, isOutput=False)
    ini_d = nc.declare_dram_parameter("ini", [nsub * P, D * F], F32, isOutput=False)
    np4_d = nc.declare_dram_parameter("np4", [nsup * P, F], F32, isOutput=False)
    idxw_d = nc.declare_dram_parameter(
        "idxw", [nsup * P, GSUB * D * P // 16], I16, isOutput=False
    )
    w4q_d = nc.declare_dram_parameter("w4q", [P, P], F32, isOutput=False)
    a4q_d = nc.declare_dram_parameter("a4q", [P, 4], F32, isOutput=False)
    o4rep_d = nc.declare_dram_parameter("o4rep", [P, P], F32, isOutput=False)
    bblk_d = nc.declare_dram_parameter("bblk", [P, 1], F32, isOutput=False)
    out_d = nc.declare_dram_parameter("out", [nsub * P, D * F], F32, isOutput=True)

    with ExitStack() as ctx:
        tc = ctx.enter_context(tile.TileContext(nc))

        dram = ctx.enter_context(tc.tile_pool(name="dram", bufs=1, space="DRAM"))
        cc_in = dram.tile([vsh, EP], F32)
        # +32 zero rows: out-of-range ids (pad) gather row `vfull` -> zeros
        summed_full = dram.tile([vfull + 32, EP], F32)

        cpool = ctx.enter_context(tc.tile_pool(name="const", bufs=1))
        w4q = cpool.tile([P, P], F32, tag="w4q")
        a4q = cpool.tile([P, 4], F32, tag="a4q")
        o4rep = cpool.tile([P, P], F32, tag="o4rep")
        bblk = cpool.tile([P, 1], F32, tag="bblk")
        nc.sync.dma_start(out=w4q[:], in_=w4q_d[:])
        nc.sync.dma_start(out=a4q[:], in_=a4q_d[:])
        nc.sync.dma_start(out=o4rep[:], in_=o4rep_d[:])
        nc.sync.dma_start(out=bblk[:], in_=bblk_d[:])

        # ---- phase 1: per-node sum over D ----
        p1pool = ctx.enter_context(tc.tile_pool(name="p1", bufs=8))
        for v0 in p1:
            xt = p1pool.tile([P, D * F], F32, tag="xt")
            sm = p1pool.tile([P, F], F32, tag="sm")
            nc.sync.dma_start(
                out=xt[:], in_=x_d[v0 : v0 + P].rearrange("v d f -> v (d f)")
            )
            # reduce over d (innermost after the view)
            nc.vector.tensor_reduce(
                out=sm[:],
                in_=xt[:].rearrange("p (d f) -> p f d", d=D),
                axis=mybir.AxisListType.X,
                op=mybir.AluOpType.add,
            )
            # the (unaligned) tail tile recomputes rows already covered by
            # the previous tile; write only the new rows so no two DMAs
            # target overlapping DRAM (overlap hangs the device).
            # both 32-col halves of the padded row get sm, so cc_in is
            # fully initialized for the collective.
            lo = 128 * (vsh // 128) if v0 == vsh - P and vsh % 128 else v0
            nc.sync.dma_start(out=cc_in[lo : v0 + P, 0:F], in_=sm[lo - v0 :, :])
            nc.sync.dma_start(out=cc_in[lo : v0 + P, F:EP], in_=sm[lo - v0 :, :])

        # zero the pad rows (gathers of out-of-range ids land here)
        zt = p1pool.tile([32, EP], F32, tag="zt")
        nc.vector.memset(zt[:], 0.0)
        nc.sync.dma_start(out=summed_full[vfull : vfull + 32, :], in_=zt[:])

        # ---- all-gather the summed table within core pairs ----
        nc.gpsimd.collective_compute(
            "AllGather",
            mybir.AluOpType.bypass,
            replica_groups=replica_groups,
            ins=[cc_in[:]],
            outs=[summed_full[0:vfull, :]],
        )

        # ---- phase 2 ----
        sb = ctx.enter_context(tc.tile_pool(name="sb", bufs=2))
        ps_y = ctx.enter_context(tc.tile_pool(name="psy", bufs=2, space="PSUM"))
        ps_a = ctx.enter_context(tc.tile_pool(name="psa", bufs=2, space="PSUM"))
        ps_s = ctx.enter_context(tc.tile_pool(name="pss", bufs=2, space="PSUM"))

        nidregs = {}
        for sp in range(nsup):
            k = int(k_sup[sp])
            if k == 0:
                continue
            kf = k * F
            nidx = GSUB * k * P
            ncol = nidx // 16
            if nidx not in nidregs:
                nidregs[nidx] = nc.gpsimd.to_reg(nidx)
            nidreg = nidregs[nidx]

            X = sb.tile([P, GSUB * kf], F32, tag="X")
            G = sb.tile([P, GSUB * k * EP], F32, tag="G")
            ixw = sb.tile([P, ncol], I16, tag="ixw")
            Xt = sb.tile([P, GSUB * kf], F32, tag="Xt")
            tb = sb.tile([P, GSUB * kf], F32, tag="tb")
            ob = sb.tile([P, GSUB * kf], F32, tag="ob")
            np4 = sb.tile([P, F], F32, tag="np4")
            E4 = sb.tile([P, kf], F32, tag="E4")
            szm = sb.tile([P, kf], F32, tag="szm")
            sum4 = sb.tile([P, F], F32, tag="sum4")
            r4 = sb.tile([P, F], F32, tag="r4")

            AT4 = ps_a.tile([P, kf], F32, tag="AT4")

            nc.vector.memset(AT4[:], 0.0)
            nc.sync.dma_start(out=ixw[:], in_=idxw_d[sp * P : (sp + 1) * P, 0:ncol])
            nc.sync.dma_start(out=np4[:], in_=np4_d[sp * P : (sp + 1) * P, :])
            for s in range(GSUB):
                gi = sp * GSUB + s
                nc.sync.dma_start(
                    out=X[:, s * kf : (s + 1) * kf],
                    in_=ini_d[gi * P : (gi + 1) * P, 0:kf],
                )
            # one batched gather for the whole super-tile:
            # G[p, c, :] = summed_full[idx_flat[c*128 + p], :], c = s*k + r
            nc.gpsimd.dma_gather(
                out_ap=G[:].rearrange("p (c e) -> p c e", e=EP),
                in_ap=summed_full[:],
                idxs_ap=ixw[:],
                num_idxs=nidx,
                num_idxs_reg=nidreg,
                elem_size=EP,
                single_packet=False,
            )
            # X += gathered (first F of each padded row)
            nc.vector.tensor_tensor(
                out=X[:].rearrange("p (c f) -> p c f", f=F),
                in0=X[:].rearrange("p (c f) -> p c f", f=F),
                in1=G[:].rearrange("p (c e) -> p c e", e=EP)[:, :, 0:F],
                op=mybir.AluOpType.add,
            )
            # block-transpose: puts (r,f) on partitions in 32-blocks
            nc.vector.transpose(out=Xt[:], in_=X[:])

            for s in range(GSUB):
                y = ps_y.tile([P, kf], F32, tag="y")
                nc.tensor.matmul(
                    out=y[:],
                    lhsT=w4q[:],
                    rhs=Xt[:, s * kf : (s + 1) * kf],
                    start=True,
                    stop=True,
                )
                nc.scalar.activation(
                    out=tb[:, s * kf : (s + 1) * kf],
                    in_=y[:],
                    func=mybir.ActivationFunctionType.Relu,
                    bias=bblk[:],
                )
                nc.tensor.matmul(
                    out=AT4[32 * s : 32 * s + 4, :],
                    lhsT=a4q[:],
                    rhs=tb[:, s * kf : (s + 1) * kf],
                    start=True,
                    stop=True,
                    tile_position=(0, 32 * s),
                )

            # softmax over r, batched over the 4 subtiles
            nc.scalar.activation(
                out=E4[:], in_=AT4[:], func=mybir.ActivationFunctionType.Exp
            )
            if k > 1:
                nc.vector.tensor_reduce(
                    out=sum4[:],
                    in_=E4[:].rearrange("p (r j) -> p j r", r=k),
                    axis=mybir.AxisListType.X,
                    op=mybir.AluOpType.add,
                )
            else:
                nc.vector.copy(out=sum4[:], in_=E4[:])
            nc.vector.tensor_tensor(
                out=sum4[:], in0=sum4[:], in1=np4[:], op=mybir.AluOpType.subtract
            )
            nc.vector.tensor_scalar_max(out=sum4[:], in0=sum4[:], scalar1=1e-30)
            nc.vector.reciprocal(out=r4[:], in_=sum4[:])
            nc.vector.tensor_tensor(
                out=szm[:].rearrange("p (r j) -> p j r", r=k),
                in0=E4[:].rearrange("p (r j) -> p j r", r=k),
                in1=r4[:].to_broadcast([P, F, k]),
                op=mybir.AluOpType.mult,
            )

            for s in range(GSUB):
                S = ps_s.tile([P, kf], F32, tag="S")
                nc.tensor.matmul(
                    out=S[:],
                    lhsT=o4rep[32 * s : 32 * s + 4, :],
                    rhs=szm[32 * s : 32 * s + 4, :],
                    start=True,
                    stop=True,
                    tile_position=(32 * s, 0),
                )
                nc.vector.tensor_tensor(
                    out=ob[:, s * kf : (s + 1) * kf],
                    in0=tb[:, s * kf : (s + 1) * kf],
                    in1=S[:],
                    op=mybir.AluOpType.mult,
                )
                gi = sp * GSUB + s
                nc.sync.dma_start(
                    out=out_d[gi * P : (gi + 1) * P, 0:kf],
                    in_=ob[:, s * kf : (s + 1) * kf],
                )

    nc.finalize()
    return nc


# ---------------- host side ----------------


def _consts(W_kernel, W_bias, a_kernel):
    w4q = np.zeros((P, P), np.float32)
    a4q = np.zeros((P, 4), np.float32)
    o4rep = np.zeros((P, P), np.float32)
    bblk = np.zeros((P, 1), np.float32)
    for g in range(4):
        w4q[32 * g : 32 * g + 32, 32 * g : 32 * g + 32] = W_kernel
        a4q[32 * g : 32 * g + 32, g] = a_kernel[:, 0]
        bblk[32 * g : 32 * g + 32, 0] = W_bias
        for s in range(4):
            o4rep[32 * s + g, 32 * g : 32 * g + 32] = 1.0
    return w4q, a4q, o4rep, bblk


def _pack_core(adj_core, mask_core, ini_core, vsh=VSH):
    """Per-core mask packing.

    Returns (node_order, dord, kv, k_sub) where
      node_order[i]   : node index at sorted position i (descending k)
      dord[v, r]      : the r-th unmasked direction of node v (orig index)
      kv[v]           : unmasked degree
      k_sub[i]        : max k in subtile i (under this core's sort)
    """
    kv = (D - mask_core.sum(axis=1)).astype(np.int64)  # [vsh]
    node_order = np.argsort(-kv, kind="stable")
    dord = np.argsort(mask_core, axis=1, kind="stable")  # unmasked first
    subs = np.array(_sub_starts(vsh))
    k_sub = kv[node_order[subs]]  # first node of each subtile = max (desc)
    return node_order, dord, kv, k_sub


def _pack_inputs(adj_core, ini_core, node_order, dord, kv, k_sup, vsh=VSH):
    """Build packed ini / idxw / np4 arrays for one core."""
    subs = np.array(_sub_starts(vsh))
    nsub = len(subs)
    nsup = nsub // GSUB
    ncol_max = GSUB * D * P // 16

    ini_p = np.zeros((nsub * P, D * F), np.float32)
    np4 = np.zeros((nsup * P, F), np.float32)
    idxw = np.zeros((nsup * P, ncol_max), np.int16)

    pvec = np.arange(P)
    for sp in range(nsup):
        k = int(k_sup[sp])
        if k == 0:
            continue
        # [GSUB, P] node ids for the 4 subtiles
        v0s = subs[sp * GSUB : (sp + 1) * GSUB]
        nodes = node_order[v0s[:, None] + pvec[None, :]]  # [4, 128]
        kvn = kv[nodes]  # [4, 128]
        rr = np.arange(k)
        real = rr[None, None, :] < kvn[:, :, None]  # [4, 128, k]
        dsel = dord[nodes][:, :, :k]  # [4, 128, k] direction of rank r
        # packed ini: [4, 128, k, F]
        ini_sel = ini_core[nodes[:, :, None], dsel]  # [4,128,k,F]
        ini_sel = np.where(real[..., None], ini_sel, 0.0).astype(np.float32)
        for s in range(GSUB):
            gi = sp * GSUB + s
            ini_p[gi * P : (gi + 1) * P, 0 : k * F] = ini_sel[s].reshape(P, k * F)
        # pad-count correction: row 32s+g, col u = k - kv(node 32g+u of sub s)
        npads = (k - kvn).astype(np.float32)  # [4, 128] = [s, 32g+u]
        for s in range(GSUB):
            np4[sp * P + 32 * s : sp * P + 32 * s + 4, :] = npads[s].reshape(4, 32)
        # gather indices: flat i = c*128 + p, c = s*k + r
        idx = adj_core[nodes[:, :, None], dsel]  # [4, 128, k]
        idx = np.where(real, idx, V)  # pads gather the zero row
        flat = idx.transpose(0, 2, 1).reshape(GSUB * k * P)  # [(s,r,p)]
        wrapped = flat.reshape(-1, 16).T  # [16, ncol]
        ncol = GSUB * k * P // 16
        idxw[sp * P : (sp + 1) * P, 0:ncol] = np.tile(wrapped, (8, 1))
    return ini_p, np4, idxw


def _unpack_core(out_dev, node_order, dord, kv, k_sup, out_full_core, vsh=VSH):
    """Block-layout packed output -> [vsh, D, F] (zeros pre-filled)."""
    subs = _sub_starts(vsh)
    nsub = len(subs)
    pvec = np.arange(P)
    for i, v0 in enumerate(subs):
        k = int(k_sup[i // GSUB])
        if k == 0:
            continue
        blk = out_dev[i * P : (i + 1) * P, 0 : k * F]
        # rows (a,o) cols (r,u): value(node 32a+u, rank r, feat o)
        blk = blk.reshape(4, 32, k, 32)  # [a, o, r, u]
        unb = blk.transpose(0, 3, 2, 1).reshape(P, k, F)  # [node, r, o]
        nodes = node_order[v0 + pvec]  # [128]
        kvn = kv[nodes]
        rr = np.arange(k)
        real = rr[None, :] < kvn[:, None]  # [128, k]
        nidx = np.repeat(nodes, k)[real.ravel()]
        didx = dord[nodes][:, :k].ravel()[real.ravel()]
        out_full_core[nidx, didx] = unb.reshape(P * k, F)[real.ravel()]


_NC_CACHE = {}


def _run(
    inputs,
    initial_states,
    mask,
    W_kernel,
    W_bias,
    a_kernel,
    adj_lst,
    mask_index,
    trace=False,
):
    from concourse.bass_utils import run_bass_kernel_spmd

    inputs = np.asarray(inputs, np.float32)
    initial_states = np.asarray(initial_states, np.float32)
    mask = np.asarray(mask, np.float32)
    adj = np.asarray(adj_lst)
    # pad ids (== mask_index) gather the zeroed pad row at V
    adj = np.where(adj == np.asarray(mask_index), V, adj).astype(np.int32)
    w4q, a4q, o4rep, bblk = _consts(
        np.asarray(W_kernel, np.float32),
        np.asarray(W_bias, np.float32),
        np.asarray(a_kernel, np.float32),
    )

    # per-core packing
    packs = []
    subs = np.array(_sub_starts(VSH))
    nsub = len(subs)
    nsup = nsub // GSUB
    k_sub_all = np.zeros((NCORES, nsub), np.int64)
    for c in range(NCORES):
        b, h = c // 2, c % 2
        sl = slice(h * VSH, (h + 1) * VSH)
        node_order, dord, kv, k_sub = _pack_core(adj[b, sl], mask[b, sl], None)
        packs.append((node_order, dord, kv))
        k_sub_all[c] = k_sub
    k_sched = k_sub_all.max(axis=0)  # same program on every core
    k_sup = np.maximum.reduce(
        [k_sched[i::GSUB][:nsup] for i in range(GSUB)]
    )

    key = tuple(int(x) for x in k_sup)
    if key not in _NC_CACHE:
        _NC_CACHE.clear()
        _NC_CACHE[key] = build_nc(k_sup)
    nc = _NC_CACHE[key]

    in_maps = []
    for c in range(NCORES):
        b, h = c // 2, c % 2
        sl = slice(h * VSH, (h + 1) * VSH)
        node_order, dord, kv = packs[c]
        ini_p, np4, idxw = _pack_inputs(
            adj[b, sl], initial_states[b, sl], node_order, dord, kv, k_sup
        )
        in_maps.append(
            {
                "x": np.ascontiguousarray(inputs[b, sl]),
                "ini": ini_p,
                "np4": np4,
                "idxw": idxw,
                "w4q": w4q,
                "a4q": a4q,
                "o4rep": o4rep,
                "bblk": bblk,
            }
        )

    res = run_bass_kernel_spmd(
        nc, in_maps, list(range(NCORES)),
        trace=trace, trace_cores=[0] if trace else None,
    )
    out = np.zeros((B, V, D, OUT), np.float32)
    for c in range(NCORES):
        b, h = c // 2, c % 2
        node_order, dord, kv = packs[c]
        _unpack_core(
            res.results[c]["out"], node_order, dord, kv, k_sup,
            out[b, h * VSH : (h + 1) * VSH],
        )
    return out, res


def kernel(
    inputs,
    initial_states,
    mask,
    W_kernel,
    W_bias,
    a_kernel,
    adj_lst,
    mask_index,
):
    out, _ = _run(
        inputs, initial_states, mask, W_kernel, W_bias, a_kernel,
        adj_lst, mask_index,
    )
    return out


# revision 10
# speedup vs baseline: 4.6081x; 1.2938x over previous
"""DirectionalGAT Trainium2 kernel (8 NeuronCores, SPMD), mask-packed.

Problem (hardcoded shapes): B=4, V=20000, D=10, F=32, OUT=32, mask_index=V.

    summed   = inputs.sum(axis=2)                      # [B,V,F]
    gathered = where(adj==V, 0, summed[b, adj])        # [B,V,D,F]
    X        = (1-mask) * (gathered + initial_states)  # [B,V,D,F]
    t        = (1-mask) * relu(X @ W + b)              # [B,V,D,OUT]
    a        = t @ a_kernel                            # [B,V,D,1]
    coefs    = softmax(a - 1e7*mask, axis=D)
    out      = coefs * t

Sharding: core c -> batch b=c//2, node half h=c%2 (Vs=10000 nodes/core).
Each core computes partial node sums for its half, an AllGather within
core pairs [2b, 2b+1] assembles the full per-batch summed table [V,F] in
DRAM, and an indirect (gather) DMA fetches the per-edge rows.

Mask packing: masked edges (mask==1) contribute exactly 0 to the output
and their gathered rows are never used, so the host drops them.  Nodes
are sorted per-core by unmasked degree k (descending); each 128-node
subtile processes only k_tile = max-k-in-tile slots per node.  Slot
(v, r) holds the node's r-th unmasked direction (host-packed ini/adj);
pad slots (r >= k_v) get ini=0 and gather the zero row, so X=0, t=0,
a=0, exp=1 -- corrected by subtracting the host-computed pad count from
the softmax denominator (exact in fp32).  The subtile slot count must be
identical across cores (SPMD single program), so k_sched[i] = max over
cores.  The host unpacks the block-layout output back to [B,V,D,F] with
zeros in masked slots.

On-chip layout per super-tile (4 subtiles batched): row-major tiles
[128 v, 4*k*F]; a DVE 32x32 block-transpose puts (r,f) on partitions in
32-blocks so a single block-diagonal weight matmul (lhsT=W4q) computes
all four 32-row v-quarters at once.  The attention dot / softmax
broadcast use static selector matmuls (a4q / o4rep) at PSUM partition
offsets 32*s so the four subtiles batch into one [128,*] region.
"""

import numpy as np
from contextlib import ExitStack

import concourse.bass as bass
import concourse.bacc as bacc
import concourse.mybir as mybir
import concourse.tile as tile

F32 = mybir.dt.float32
I32 = mybir.dt.int32
I16 = mybir.dt.int16
EP = 64  # gather table row padding (dma_gather elem must be a 256B multiple)

B, V, D, F, OUT = 4, 20000, 10, 32, 32
P = 128
NCORES = 8
VSH = V // 2        # 10000 nodes per core
GSUB = 4            # 128-v subtiles batched per super-tile
NCHUNK = 4          # collective chunks (table is chunk-major, see _remap_rows)
CROWS = VSH // NCHUNK


def _remap_rows(u):
    """Global node id -> chunk-major summed_full row.

    summed_full = [chunk 0: rank0 rows 0..CROWS | rank1 rows 0..CROWS]
                  [chunk 1: ...] ... ; pad zero rows at V unchanged."""
    h = u // VSH
    ul = u % VSH
    q = ul // CROWS
    r = ul % CROWS
    return np.where(u >= V, u, q * 2 * CROWS + h * CROWS + r)


def _sub_starts(vsh):
    """Start rows of the 128-v subtiles, padded to a multiple of GSUB subs.

    Tail subs clamp to vsh-128 (recompute overlap; duplicate subs write
    identical values to their own output rows)."""
    n = -(-vsh // P)            # ceil
    n = -(-n // GSUB) * GSUB    # pad to multiple of GSUB
    return [min(P * i, vsh - P) for i in range(n)]


def build_nc(k_sup, vsh=VSH, vfull=V, num_devices=NCORES, replica_groups=None):
    """Build the Bass program for one core (SPMD-identical across cores).

    k_sup: per-super-tile slot count (same for all cores)."""
    if replica_groups is None:
        replica_groups = [[2 * b, 2 * b + 1] for b in range(num_devices // 2)]
    subs = _sub_starts(vsh)
    nsub = len(subs)
    nsup = nsub // GSUB
    assert len(k_sup) == nsup
    # phase-1 tiles: 256 node rows each (2 per partition), overlap tail
    P2 = 2 * P
    n1 = vsh // P2
    p1 = [P2 * i for i in range(n1)]
    if n1 * P2 != vsh:
        p1.append(vsh - P2)

    nc = bacc.Bacc("TRN2", num_devices=num_devices)

    x_d = nc.declare_dram_parameter("x", [vsh, D, F], F32, isOutput=False)
    ini_d = nc.declare_dram_parameter("ini", [nsub * P, D * F], F32, isOutput=False)
    np4_d = nc.declare_dram_parameter("np4", [nsup * P, F], F32, isOutput=False)
    idxw_d = nc.declare_dram_parameter(
        "idxw", [nsup * P, GSUB * D * P // 16], I16, isOutput=False
    )
    w4q_d = nc.declare_dram_parameter("w4q", [P, P], F32, isOutput=False)
    a4q_d = nc.declare_dram_parameter("a4q", [P, 4], F32, isOutput=False)
    o4rep_d = nc.declare_dram_parameter("o4rep", [P, P], F32, isOutput=False)
    bblk_d = nc.declare_dram_parameter("bblk", [P, 1], F32, isOutput=False)
    out_d = nc.declare_dram_parameter("out", [nsub * P, D * F], F32, isOutput=True)

    with ExitStack() as ctx:
        tc = ctx.enter_context(tile.TileContext(nc))

        dram = ctx.enter_context(tc.tile_pool(name="dram", bufs=1, space="DRAM"))
        cc_in = dram.tile([vsh, EP], F32)
        # +32 zero rows: out-of-range ids (pad) gather row `vfull` -> zeros
        summed_full = dram.tile([vfull + 32, EP], F32)

        cpool = ctx.enter_context(tc.tile_pool(name="const", bufs=1))
        w4q = cpool.tile([P, P], F32, tag="w4q")
        a4q = cpool.tile([P, 4], F32, tag="a4q")
        o4rep = cpool.tile([P, P], F32, tag="o4rep")
        bblk = cpool.tile([P, 1], F32, tag="bblk")
        nc.sync.dma_start(out=w4q[:], in_=w4q_d[:])
        nc.sync.dma_start(out=a4q[:], in_=a4q_d[:])
        nc.sync.dma_start(out=o4rep[:], in_=o4rep_d[:])
        nc.sync.dma_start(out=bblk[:], in_=bblk_d[:])

        # ---- phase 1: per-node sum over D ----
        # 256 node rows per tile: partition p holds rows v0+2p, v0+2p+1.
        p1pool = ctx.enter_context(tc.tile_pool(name="p1", bufs=8))
        for v0 in p1:
            xt = p1pool.tile([P, 2 * D * F], F32, tag="xt")
            sm = p1pool.tile([P, 2 * EP], F32, tag="sm")
            nc.sync.dma_start(
                out=xt[:],
                in_=x_d[v0 : v0 + 2 * P].rearrange("(p v2) d f -> p (v2 d f)", v2=2),
            )
            nc.vector.memset(sm[:], 0.0)
            # reduce over d (innermost after the view) into cols 0:F of
            # each EP-wide half-row
            nc.vector.tensor_reduce(
                out=sm[:].rearrange("p (v2 e) -> p v2 e", v2=2)[:, :, 0:F],
                in_=xt[:].rearrange("p (v2 d f) -> p v2 f d", v2=2, d=D),
                axis=mybir.AxisListType.X,
                op=mybir.AluOpType.add,
            )
            # the (unaligned) tail tile recomputes rows already covered by
            # the previous tile; write only the new rows so no two DMAs
            # target overlapping DRAM (overlap hangs the device).
            lo = 2 * P * (vsh // (2 * P)) if v0 == vsh - 2 * P and vsh % (2 * P) else v0
            # new rows start at partition (lo-v0)//2 (lo-v0 is even)
            po = (lo - v0) // 2
            nc.sync.dma_start(
                out=cc_in[lo : v0 + 2 * P].rearrange("(p v2) e -> p (v2 e)", v2=2),
                in_=sm[po:, :],
            )

        # zero the pad rows (gathers of out-of-range ids land here)
        zt = p1pool.tile([32, EP], F32, tag="zt")
        nc.vector.memset(zt[:], 0.0)
        nc.sync.dma_start(out=summed_full[vfull : vfull + 32, :], in_=zt[:])

        # ---- all-gather the summed table within core pairs (chunked;
        # chunk-major table layout, see _remap_rows) ----
        crows = vsh // NCHUNK
        for q in range(NCHUNK):
            nc.gpsimd.collective_compute(
                "AllGather",
                mybir.AluOpType.bypass,
                replica_groups=replica_groups,
                ins=[cc_in[q * crows : (q + 1) * crows, :]],
                outs=[summed_full[2 * q * crows : 2 * (q + 1) * crows, :]],
            )

        # ---- phase 2 ----
        sb = ctx.enter_context(tc.tile_pool(name="sb", bufs=3))
        ps_y = ctx.enter_context(tc.tile_pool(name="psy", bufs=2, space="PSUM"))
        ps_a = ctx.enter_context(tc.tile_pool(name="psa", bufs=2, space="PSUM"))
        ps_s = ctx.enter_context(tc.tile_pool(name="pss", bufs=2, space="PSUM"))

        nidregs = {}
        for sp in range(nsup):
            k = int(k_sup[sp])
            if k == 0:
                continue
            kf = k * F
            nidx = GSUB * k * P
            ncol = nidx // 16
            if nidx not in nidregs:
                nidregs[nidx] = nc.gpsimd.to_reg(nidx)
            nidreg = nidregs[nidx]

            X = sb.tile([P, GSUB * kf], F32, tag="X")
            G = sb.tile([P, GSUB * k * EP], F32, tag="G")
            ixw = sb.tile([P, ncol], I16, tag="ixw")
            Xt = sb.tile([P, GSUB * kf], F32, tag="Xt")
            tb = sb.tile([P, GSUB * kf], F32, tag="tb")
            ob = sb.tile([P, GSUB * kf], F32, tag="ob")
            np4 = sb.tile([P, F], F32, tag="np4")
            E4 = sb.tile([P, kf], F32, tag="E4")
            szm = sb.tile([P, kf], F32, tag="szm")
            sum4 = sb.tile([P, F], F32, tag="sum4")
            r4 = sb.tile([P, F], F32, tag="r4")

            AT4 = ps_a.tile([P, kf], F32, tag="AT4")

            nc.vector.memset(AT4[:], 0.0)
            nc.sync.dma_start(out=ixw[:], in_=idxw_d[sp * P : (sp + 1) * P, 0:ncol])
            nc.sync.dma_start(out=np4[:], in_=np4_d[sp * P : (sp + 1) * P, :])
            for s in range(GSUB):
                gi = sp * GSUB + s
                nc.sync.dma_start(
                    out=X[:, s * kf : (s + 1) * kf],
                    in_=ini_d[gi * P : (gi + 1) * P, 0:kf],
                )
            # one batched gather for the whole super-tile:
            # G[p, c, :] = summed_full[idx_flat[c*128 + p], :], c = s*k + r
            nc.gpsimd.dma_gather(
                out_ap=G[:].rearrange("p (c e) -> p c e", e=EP),
                in_ap=summed_full[:],
                idxs_ap=ixw[:],
                num_idxs=nidx,
                num_idxs_reg=nidreg,
                elem_size=EP,
                single_packet=False,
            )
            # X += gathered (first F of each padded row)
            nc.vector.tensor_tensor(
                out=X[:].rearrange("p (c f) -> p c f", f=F),
                in0=X[:].rearrange("p (c f) -> p c f", f=F),
                in1=G[:].rearrange("p (c e) -> p c e", e=EP)[:, :, 0:F],
                op=mybir.AluOpType.add,
            )
            # block-transpose: puts (r,f) on partitions in 32-blocks
            nc.vector.transpose(out=Xt[:], in_=X[:])

            for s in range(GSUB):
                y = ps_y.tile([P, kf], F32, tag="y")
                nc.tensor.matmul(
                    out=y[:],
                    lhsT=w4q[:],
                    rhs=Xt[:, s * kf : (s + 1) * kf],
                    start=True,
                    stop=True,
                )
                nc.scalar.activation(
                    out=tb[:, s * kf : (s + 1) * kf],
                    in_=y[:],
                    func=mybir.ActivationFunctionType.Relu,
                    bias=bblk[:],
                )
                nc.tensor.matmul(
                    out=AT4[32 * s : 32 * s + 4, :],
                    lhsT=a4q[:],
                    rhs=tb[:, s * kf : (s + 1) * kf],
                    start=True,
                    stop=True,
                    tile_position=(0, 32 * s),
                )

            # softmax over r, batched over the 4 subtiles
            nc.scalar.activation(
                out=E4[:], in_=AT4[:], func=mybir.ActivationFunctionType.Exp
            )
            if k > 1:
                nc.vector.tensor_reduce(
                    out=sum4[:],
                    in_=E4[:].rearrange("p (r j) -> p j r", r=k),
                    axis=mybir.AxisListType.X,
                    op=mybir.AluOpType.add,
                )
            else:
                nc.vector.copy(out=sum4[:], in_=E4[:])
            nc.vector.tensor_tensor(
                out=sum4[:], in0=sum4[:], in1=np4[:], op=mybir.AluOpType.subtract
            )
            nc.vector.tensor_scalar_max(out=sum4[:], in0=sum4[:], scalar1=1e-30)
            nc.vector.reciprocal(out=r4[:], in_=sum4[:])
            nc.vector.tensor_tensor(
                out=szm[:].rearrange("p (r j) -> p j r", r=k),
                in0=E4[:].rearrange("p (r j) -> p j r", r=k),
                in1=r4[:].to_broadcast([P, F, k]),
                op=mybir.AluOpType.mult,
            )

            for s in range(GSUB):
                S = ps_s.tile([P, kf], F32, tag="S")
                nc.tensor.matmul(
                    out=S[:],
                    lhsT=o4rep[32 * s : 32 * s + 4, :],
                    rhs=szm[32 * s : 32 * s + 4, :],
                    start=True,
                    stop=True,
                    tile_position=(32 * s, 0),
                )
                nc.vector.tensor_tensor(
                    out=ob[:, s * kf : (s + 1) * kf],
                    in0=tb[:, s * kf : (s + 1) * kf],
                    in1=S[:],
                    op=mybir.AluOpType.mult,
                )
                gi = sp * GSUB + s
                nc.sync.dma_start(
                    out=out_d[gi * P : (gi + 1) * P, 0:kf],
                    in_=ob[:, s * kf : (s + 1) * kf],
                )

    nc.finalize()
    return nc


# ---------------- host side ----------------


def _consts(W_kernel, W_bias, a_kernel):
    w4q = np.zeros((P, P), np.float32)
    a4q = np.zeros((P, 4), np.float32)
    o4rep = np.zeros((P, P), np.float32)
    bblk = np.zeros((P, 1), np.float32)
    for g in range(4):
        w4q[32 * g : 32 * g + 32, 32 * g : 32 * g + 32] = W_kernel
        a4q[32 * g : 32 * g + 32, g] = a_kernel[:, 0]
        bblk[32 * g : 32 * g + 32, 0] = W_bias
        for s in range(4):
            o4rep[32 * s + g, 32 * g : 32 * g + 32] = 1.0
    return w4q, a4q, o4rep, bblk


def _pack_core(adj_core, mask_core, ini_core, vsh=VSH):
    """Per-core mask packing.

    Returns (node_order, dord, kv, k_sub) where
      node_order[i]   : node index at sorted position i (descending k)
      dord[v, r]      : the r-th unmasked direction of node v (orig index)
      kv[v]           : unmasked degree
      k_sub[i]        : max k in subtile i (under this core's sort)
    """
    kv = (D - mask_core.sum(axis=1)).astype(np.int64)  # [vsh]
    node_order = np.argsort(-kv, kind="stable")
    dord = np.argsort(mask_core, axis=1, kind="stable")  # unmasked first
    subs = np.array(_sub_starts(vsh))
    k_sub = kv[node_order[subs]]  # first node of each subtile = max (desc)
    return node_order, dord, kv, k_sub


def _pack_inputs(adj_core, ini_core, node_order, dord, kv, k_sup, vsh=VSH):
    """Build packed ini / idxw / np4 arrays for one core."""
    subs = np.array(_sub_starts(vsh))
    nsub = len(subs)
    nsup = nsub // GSUB
    ncol_max = GSUB * D * P // 16

    ini_p = np.zeros((nsub * P, D * F), np.float32)
    np4 = np.zeros((nsup * P, F), np.float32)
    idxw = np.zeros((nsup * P, ncol_max), np.int16)

    pvec = np.arange(P)
    for sp in range(nsup):
        k = int(k_sup[sp])
        if k == 0:
            continue
        # [GSUB, P] node ids for the 4 subtiles
        v0s = subs[sp * GSUB : (sp + 1) * GSUB]
        nodes = node_order[v0s[:, None] + pvec[None, :]]  # [4, 128]
        kvn = kv[nodes]  # [4, 128]
        rr = np.arange(k)
        real = rr[None, None, :] < kvn[:, :, None]  # [4, 128, k]
        dsel = dord[nodes][:, :, :k]  # [4, 128, k] direction of rank r
        # packed ini: [4, 128, k, F]
        ini_sel = ini_core[nodes[:, :, None], dsel]  # [4,128,k,F]
        ini_sel = np.where(real[..., None], ini_sel, 0.0).astype(np.float32)
        for s in range(GSUB):
            gi = sp * GSUB + s
            ini_p[gi * P : (gi + 1) * P, 0 : k * F] = ini_sel[s].reshape(P, k * F)
        # pad-count correction: row 32s+g, col u = k - kv(node 32g+u of sub s)
        npads = (k - kvn).astype(np.float32)  # [4, 128] = [s, 32g+u]
        for s in range(GSUB):
            np4[sp * P + 32 * s : sp * P + 32 * s + 4, :] = npads[s].reshape(4, 32)
        # gather indices: flat i = c*128 + p, c = s*k + r
        idx = adj_core[nodes[:, :, None], dsel]  # [4, 128, k]
        idx = np.where(real, idx, V)  # pads gather the zero row
        idx = _remap_rows(idx)  # chunk-major table layout
        flat = idx.transpose(0, 2, 1).reshape(GSUB * k * P)  # [(s,r,p)]
        wrapped = flat.reshape(-1, 16).T  # [16, ncol]
        ncol = GSUB * k * P // 16
        idxw[sp * P : (sp + 1) * P, 0:ncol] = np.tile(wrapped, (8, 1))
    return ini_p, np4, idxw


def _unpack_core(out_dev, node_order, dord, kv, k_sup, out_full_core, vsh=VSH):
    """Block-layout packed output -> [vsh, D, F] (zeros pre-filled)."""
    subs = _sub_starts(vsh)
    nsub = len(subs)
    pvec = np.arange(P)
    for i, v0 in enumerate(subs):
        k = int(k_sup[i // GSUB])
        if k == 0:
            continue
        blk = out_dev[i * P : (i + 1) * P, 0 : k * F]
        # rows (a,o) cols (r,u): value(node 32a+u, rank r, feat o)
        blk = blk.reshape(4, 32, k, 32)  # [a, o, r, u]
        unb = blk.transpose(0, 3, 2, 1).reshape(P, k, F)  # [node, r, o]
        nodes = node_order[v0 + pvec]  # [128]
        kvn = kv[nodes]
        rr = np.arange(k)
        real = rr[None, :] < kvn[:, None]  # [128, k]
        nidx = np.repeat(nodes, k)[real.ravel()]
        didx = dord[nodes][:, :k].ravel()[real.ravel()]
        out_full_core[nidx, didx] = unb.reshape(P * k, F)[real.ravel()]


_NC_CACHE = {}


def _run(
    inputs,
    initial_states,
    mask,
    W_kernel,
    W_bias,
    a_kernel,
    adj_lst,
    mask_index,
    trace=False,
):
    from concourse.bass_utils import run_bass_kernel_spmd

    inputs = np.asarray(inputs, np.float32)
    initial_states = np.asarray(initial_states, np.float32)
    mask = np.asarray(mask, np.float32)
    adj = np.asarray(adj_lst)
    # pad ids (== mask_index) gather the zeroed pad row at V
    adj = np.where(adj == np.asarray(mask_index), V, adj).astype(np.int32)
    w4q, a4q, o4rep, bblk = _consts(
        np.asarray(W_kernel, np.float32),
        np.asarray(W_bias, np.float32),
        np.asarray(a_kernel, np.float32),
    )

    # per-core packing
    packs = []
    subs = np.array(_sub_starts(VSH))
    nsub = len(subs)
    nsup = nsub // GSUB
    k_sub_all = np.zeros((NCORES, nsub), np.int64)
    for c in range(NCORES):
        b, h = c // 2, c % 2
        sl = slice(h * VSH, (h + 1) * VSH)
        node_order, dord, kv, k_sub = _pack_core(adj[b, sl], mask[b, sl], None)
        packs.append((node_order, dord, kv))
        k_sub_all[c] = k_sub
    k_sched = k_sub_all.max(axis=0)  # same program on every core
    k_sup = np.maximum.reduce(
        [k_sched[i::GSUB][:nsup] for i in range(GSUB)]
    )

    key = tuple(int(x) for x in k_sup)
    if key not in _NC_CACHE:
        _NC_CACHE.clear()
        _NC_CACHE[key] = build_nc(k_sup)
    nc = _NC_CACHE[key]

    in_maps = []
    for c in range(NCORES):
        b, h = c // 2, c % 2
        sl = slice(h * VSH, (h + 1) * VSH)
        node_order, dord, kv = packs[c]
        ini_p, np4, idxw = _pack_inputs(
            adj[b, sl], initial_states[b, sl], node_order, dord, kv, k_sup
        )
        in_maps.append(
            {
                "x": np.ascontiguousarray(inputs[b, sl]),
                "ini": ini_p,
                "np4": np4,
                "idxw": idxw,
                "w4q": w4q,
                "a4q": a4q,
                "o4rep": o4rep,
                "bblk": bblk,
            }
        )

    res = run_bass_kernel_spmd(
        nc, in_maps, list(range(NCORES)),
        trace=trace, trace_cores=[0] if trace else None,
    )
    out = np.zeros((B, V, D, OUT), np.float32)
    for c in range(NCORES):
        b, h = c // 2, c % 2
        node_order, dord, kv = packs[c]
        _unpack_core(
            res.results[c]["out"], node_order, dord, kv, k_sup,
            out[b, h * VSH : (h + 1) * VSH],
        )
    return out, res


def kernel(
    inputs,
    initial_states,
    mask,
    W_kernel,
    W_bias,
    a_kernel,
    adj_lst,
    mask_index,
):
    out, _ = _run(
        inputs, initial_states, mask, W_kernel, W_bias, a_kernel,
        adj_lst, mask_index,
    )
    return out


# revision 12
# speedup vs baseline: 4.6167x; 1.0019x over previous
"""DirectionalGAT Trainium2 kernel (8 NeuronCores, SPMD), mask-packed.

Problem (hardcoded shapes): B=4, V=20000, D=10, F=32, OUT=32, mask_index=V.

    summed   = inputs.sum(axis=2)                      # [B,V,F]
    gathered = where(adj==V, 0, summed[b, adj])        # [B,V,D,F]
    X        = (1-mask) * (gathered + initial_states)  # [B,V,D,F]
    t        = (1-mask) * relu(X @ W + b)              # [B,V,D,OUT]
    a        = t @ a_kernel                            # [B,V,D,1]
    coefs    = softmax(a - 1e7*mask, axis=D)
    out      = coefs * t

Sharding: core c -> batch b=c//2, node half h=c%2 (Vs=10000 nodes/core).
Each core computes partial node sums for its half, an AllGather within
core pairs [2b, 2b+1] assembles the full per-batch summed table [V,F] in
DRAM, and an indirect (gather) DMA fetches the per-edge rows.

Mask packing: masked edges (mask==1) contribute exactly 0 to the output
and their gathered rows are never used, so the host drops them.  Nodes
are sorted per-core by unmasked degree k (descending); each 128-node
subtile processes only k_tile = max-k-in-tile slots per node.  Slot
(v, r) holds the node's r-th unmasked direction (host-packed ini/adj);
pad slots (r >= k_v) get ini=0 and gather the zero row, so X=0, t=0,
a=0, exp=1 -- corrected by subtracting the host-computed pad count from
the softmax denominator (exact in fp32).  The subtile slot count must be
identical across cores (SPMD single program), so k_sched[i] = max over
cores.  The host unpacks the block-layout output back to [B,V,D,F] with
zeros in masked slots.

On-chip layout per super-tile (4 subtiles batched): row-major tiles
[128 v, 4*k*F]; a DVE 32x32 block-transpose puts (r,f) on partitions in
32-blocks so a single block-diagonal weight matmul (lhsT=W4q) computes
all four 32-row v-quarters at once.  The attention dot / softmax
broadcast use static selector matmuls (a4q / o4rep) at PSUM partition
offsets 32*s so the four subtiles batch into one [128,*] region.
"""

import numpy as np
from contextlib import ExitStack

import concourse.bass as bass
import concourse.bacc as bacc
import concourse.mybir as mybir
import concourse.tile as tile

F32 = mybir.dt.float32
I32 = mybir.dt.int32
I16 = mybir.dt.int16
EP = 64  # gather table row padding (dma_gather elem must be a 256B multiple)

B, V, D, F, OUT = 4, 20000, 10, 32, 32
P = 128
NCORES = 8
VSH = V // 2        # 10000 nodes per core
GSUB = 4            # 128-v subtiles batched per super-tile
NCHUNK = 4          # collective chunks (table is chunk-major, see _remap_rows)
CROWS = VSH // NCHUNK


def _remap_rows(u):
    """Global node id -> chunk-major summed_full row.

    summed_full = [chunk 0: rank0 rows 0..CROWS | rank1 rows 0..CROWS]
                  [chunk 1: ...] ... ; pad zero rows at V unchanged."""
    h = u // VSH
    ul = u % VSH
    q = ul // CROWS
    r = ul % CROWS
    return np.where(u >= V, u, q * 2 * CROWS + h * CROWS + r)


def _sub_starts(vsh):
    """Start rows of the 128-v subtiles, padded to a multiple of GSUB subs.

    Tail subs clamp to vsh-128 (recompute overlap; duplicate subs write
    identical values to their own output rows)."""
    n = -(-vsh // P)            # ceil
    n = -(-n // GSUB) * GSUB    # pad to multiple of GSUB
    return [min(P * i, vsh - P) for i in range(n)]


def build_nc(k_sup, vsh=VSH, vfull=V, num_devices=NCORES, replica_groups=None):
    """Build the Bass program for one core (SPMD-identical across cores).

    k_sup: per-super-tile slot count (same for all cores)."""
    if replica_groups is None:
        replica_groups = [[2 * b, 2 * b + 1] for b in range(num_devices // 2)]
    subs = _sub_starts(vsh)
    nsub = len(subs)
    nsup = nsub // GSUB
    assert len(k_sup) == nsup
    # phase-1 tiles: 256 node rows each (2 per partition), overlap tail
    P2 = 2 * P
    n1 = vsh // P2
    p1 = [P2 * i for i in range(n1)]
    if n1 * P2 != vsh:
        p1.append(vsh - P2)

    nc = bacc.Bacc("TRN2", num_devices=num_devices)

    x_d = nc.declare_dram_parameter("x", [vsh, D, F], F32, isOutput=False)
    ini_d = nc.declare_dram_parameter("ini", [nsub * P, D * F], F32, isOutput=False)
    np4_d = nc.declare_dram_parameter("np4", [nsup * P, F], F32, isOutput=False)
    idxw_d = nc.declare_dram_parameter(
        "idxw", [nsup * P, GSUB * D * P // 16], I16, isOutput=False
    )
    w4q_d = nc.declare_dram_parameter("w4q", [P, P], F32, isOutput=False)
    a4q_d = nc.declare_dram_parameter("a4q", [P, 4], F32, isOutput=False)
    o4rep_d = nc.declare_dram_parameter("o4rep", [P, P], F32, isOutput=False)
    bblk_d = nc.declare_dram_parameter("bblk", [P, 1], F32, isOutput=False)
    out_d = nc.declare_dram_parameter("out", [nsub * P, D * F], F32, isOutput=True)

    with ExitStack() as ctx:
        tc = ctx.enter_context(tile.TileContext(nc))

        dram = ctx.enter_context(tc.tile_pool(name="dram", bufs=1, space="DRAM"))
        cc_in = dram.tile([vsh, EP], F32)
        # +32 zero rows: out-of-range ids (pad) gather row `vfull` -> zeros
        summed_full = dram.tile([vfull + 32, EP], F32)

        cpool = ctx.enter_context(tc.tile_pool(name="const", bufs=1))
        w4q = cpool.tile([P, P], F32, tag="w4q")
        a4q = cpool.tile([P, 4], F32, tag="a4q")
        o4rep = cpool.tile([P, P], F32, tag="o4rep")
        bblk = cpool.tile([P, 1], F32, tag="bblk")
        nc.sync.dma_start(out=w4q[:], in_=w4q_d[:])
        nc.sync.dma_start(out=a4q[:], in_=a4q_d[:])
        nc.sync.dma_start(out=o4rep[:], in_=o4rep_d[:])
        nc.sync.dma_start(out=bblk[:], in_=bblk_d[:])

        # ---- phase 1: per-node sum over D ----
        # 256 node rows per tile: partition p holds rows v0+2p, v0+2p+1.
        p1pool = ctx.enter_context(tc.tile_pool(name="p1", bufs=8))
        for v0 in p1:
            xt = p1pool.tile([P, 2 * D * F], F32, tag="xt")
            sm = p1pool.tile([P, 2 * EP], F32, tag="sm")
            nc.sync.dma_start(
                out=xt[:],
                in_=x_d[v0 : v0 + 2 * P].rearrange("(p v2) d f -> p (v2 d f)", v2=2),
            )
            nc.vector.memset(sm[:], 0.0)
            # reduce over d (innermost after the view) into cols 0:F of
            # each EP-wide half-row
            nc.vector.tensor_reduce(
                out=sm[:].rearrange("p (v2 e) -> p v2 e", v2=2)[:, :, 0:F],
                in_=xt[:].rearrange("p (v2 d f) -> p v2 f d", v2=2, d=D),
                axis=mybir.AxisListType.X,
                op=mybir.AluOpType.add,
            )
            # the (unaligned) tail tile recomputes rows already covered by
            # the previous tile; write only the new rows so no two DMAs
            # target overlapping DRAM (overlap hangs the device).
            lo = 2 * P * (vsh // (2 * P)) if v0 == vsh - 2 * P and vsh % (2 * P) else v0
            # new rows start at partition (lo-v0)//2 (lo-v0 is even)
            po = (lo - v0) // 2
            nc.sync.dma_start(
                out=cc_in[lo : v0 + 2 * P].rearrange("(p v2) e -> p (v2 e)", v2=2),
                in_=sm[po:, :],
            )

        # zero the pad rows (gathers of out-of-range ids land here)
        zt = p1pool.tile([32, EP], F32, tag="zt")
        nc.vector.memset(zt[:], 0.0)
        nc.sync.dma_start(out=summed_full[vfull : vfull + 32, :], in_=zt[:])

        # ---- all-gather the summed table within core pairs (chunked;
        # chunk-major table layout, see _remap_rows) ----
        crows = vsh // NCHUNK
        for q in range(NCHUNK):
            nc.gpsimd.collective_compute(
                "AllGather",
                mybir.AluOpType.bypass,
                replica_groups=replica_groups,
                ins=[cc_in[q * crows : (q + 1) * crows, :]],
                outs=[summed_full[2 * q * crows : 2 * (q + 1) * crows, :]],
            )

        # ---- phase 2 ----
        sb = ctx.enter_context(tc.tile_pool(name="sb", bufs=3))
        sbi = ctx.enter_context(tc.tile_pool(name="sbi", bufs=8))
        ps_y = ctx.enter_context(tc.tile_pool(name="psy", bufs=2, space="PSUM"))
        ps_a = ctx.enter_context(tc.tile_pool(name="psa", bufs=2, space="PSUM"))
        ps_s = ctx.enter_context(tc.tile_pool(name="pss", bufs=2, space="PSUM"))

        nidregs = {}
        for sp in range(nsup):
            k = int(k_sup[sp])
            if k == 0:
                continue
            kf = k * F
            nidx = GSUB * k * P
            ncol = nidx // 16
            if nidx not in nidregs:
                nidregs[nidx] = nc.gpsimd.to_reg(nidx)
            nidreg = nidregs[nidx]

            X = sb.tile([P, GSUB * kf], F32, tag="X")
            G = sb.tile([P, GSUB * k * EP], F32, tag="G")
            ixw = sbi.tile([P, ncol], I16, tag="ixw")
            Xt = sb.tile([P, GSUB * kf], F32, tag="Xt")
            tb = sb.tile([P, GSUB * kf], F32, tag="tb")
            ob = sb.tile([P, GSUB * kf], F32, tag="ob")
            np4 = sbi.tile([P, F], F32, tag="np4")
            E4 = sb.tile([P, kf], F32, tag="E4")
            szm = sb.tile([P, kf], F32, tag="szm")
            sum4 = sb.tile([P, F], F32, tag="sum4")
            r4 = sb.tile([P, F], F32, tag="r4")

            AT4 = ps_a.tile([P, kf], F32, tag="AT4")

            nc.vector.memset(AT4[:], 0.0)
            nc.sync.dma_start(out=ixw[:], in_=idxw_d[sp * P : (sp + 1) * P, 0:ncol])
            nc.sync.dma_start(out=np4[:], in_=np4_d[sp * P : (sp + 1) * P, :])
            for s in range(GSUB):
                gi = sp * GSUB + s
                nc.sync.dma_start(
                    out=X[:, s * kf : (s + 1) * kf],
                    in_=ini_d[gi * P : (gi + 1) * P, 0:kf],
                )
            # one batched gather for the whole super-tile:
            # G[p, c, :] = summed_full[idx_flat[c*128 + p], :], c = s*k + r
            nc.gpsimd.dma_gather(
                out_ap=G[:].rearrange("p (c e) -> p c e", e=EP),
                in_ap=summed_full[:],
                idxs_ap=ixw[:],
                num_idxs=nidx,
                num_idxs_reg=nidreg,
                elem_size=EP,
                single_packet=False,
            )
            # X += gathered (first F of each padded row)
            nc.vector.tensor_tensor(
                out=X[:].rearrange("p (c f) -> p c f", f=F),
                in0=X[:].rearrange("p (c f) -> p c f", f=F),
                in1=G[:].rearrange("p (c e) -> p c e", e=EP)[:, :, 0:F],
                op=mybir.AluOpType.add,
            )
            # block-transpose: puts (r,f) on partitions in 32-blocks
            nc.vector.transpose(out=Xt[:], in_=X[:])

            for s in range(GSUB):
                y = ps_y.tile([P, kf], F32, tag="y")
                nc.tensor.matmul(
                    out=y[:],
                    lhsT=w4q[:],
                    rhs=Xt[:, s * kf : (s + 1) * kf],
                    start=True,
                    stop=True,
                )
                nc.scalar.activation(
                    out=tb[:, s * kf : (s + 1) * kf],
                    in_=y[:],
                    func=mybir.ActivationFunctionType.Relu,
                    bias=bblk[:],
                )
                nc.tensor.matmul(
                    out=AT4[32 * s : 32 * s + 4, :],
                    lhsT=a4q[:],
                    rhs=tb[:, s * kf : (s + 1) * kf],
                    start=True,
                    stop=True,
                    tile_position=(0, 32 * s),
                )

            # softmax over r, batched over the 4 subtiles
            nc.scalar.activation(
                out=E4[:], in_=AT4[:], func=mybir.ActivationFunctionType.Exp
            )
            if k > 1:
                nc.vector.tensor_reduce(
                    out=sum4[:],
                    in_=E4[:].rearrange("p (r j) -> p j r", r=k),
                    axis=mybir.AxisListType.X,
                    op=mybir.AluOpType.add,
                )
            else:
                nc.vector.copy(out=sum4[:], in_=E4[:])
            nc.vector.tensor_tensor(
                out=sum4[:], in0=sum4[:], in1=np4[:], op=mybir.AluOpType.subtract
            )
            nc.vector.tensor_scalar_max(out=sum4[:], in0=sum4[:], scalar1=1e-30)
            nc.vector.reciprocal(out=r4[:], in_=sum4[:])
            nc.vector.tensor_tensor(
                out=szm[:].rearrange("p (r j) -> p j r", r=k),
                in0=E4[:].rearrange("p (r j) -> p j r", r=k),
                in1=r4[:].to_broadcast([P, F, k]),
                op=mybir.AluOpType.mult,
            )

            for s in range(GSUB):
                S = ps_s.tile([P, kf], F32, tag="S")
                nc.tensor.matmul(
                    out=S[:],
                    lhsT=o4rep[32 * s : 32 * s + 4, :],
                    rhs=szm[32 * s : 32 * s + 4, :],
                    start=True,
                    stop=True,
                    tile_position=(32 * s, 0),
                )
                nc.vector.tensor_tensor(
                    out=ob[:, s * kf : (s + 1) * kf],
                    in0=tb[:, s * kf : (s + 1) * kf],
                    in1=S[:],
                    op=mybir.AluOpType.mult,
                )
                gi = sp * GSUB + s
                nc.sync.dma_start(
                    out=out_d[gi * P : (gi + 1) * P, 0:kf],
                    in_=ob[:, s * kf : (s + 1) * kf],
                )

    nc.finalize()
    return nc


# ---------------- host side ----------------


def _consts(W_kernel, W_bias, a_kernel):
    w4q = np.zeros((P, P), np.float32)
    a4q = np.zeros((P, 4), np.float32)
    o4rep = np.zeros((P, P), np.float32)
    bblk = np.zeros((P, 1), np.float32)
    for g in range(4):
        w4q[32 * g : 32 * g + 32, 32 * g : 32 * g + 32] = W_kernel
        a4q[32 * g : 32 * g + 32, g] = a_kernel[:, 0]
        bblk[32 * g : 32 * g + 32, 0] = W_bias
        for s in range(4):
            o4rep[32 * s + g, 32 * g : 32 * g + 32] = 1.0
    return w4q, a4q, o4rep, bblk


def _pack_core(adj_core, mask_core, ini_core, vsh=VSH):
    """Per-core mask packing.

    Returns (node_order, dord, kv, k_sub) where
      node_order[i]   : node index at sorted position i (descending k)
      dord[v, r]      : the r-th unmasked direction of node v (orig index)
      kv[v]           : unmasked degree
      k_sub[i]        : max k in subtile i (under this core's sort)
    """
    kv = (D - mask_core.sum(axis=1)).astype(np.int64)  # [vsh]
    node_order = np.argsort(-kv, kind="stable")
    dord = np.argsort(mask_core, axis=1, kind="stable")  # unmasked first
    subs = np.array(_sub_starts(vsh))
    k_sub = kv[node_order[subs]]  # first node of each subtile = max (desc)
    return node_order, dord, kv, k_sub


def _pack_inputs(adj_core, ini_core, node_order, dord, kv, k_sup, vsh=VSH):
    """Build packed ini / idxw / np4 arrays for one core."""
    subs = np.array(_sub_starts(vsh))
    nsub = len(subs)
    nsup = nsub // GSUB
    ncol_max = GSUB * D * P // 16

    ini_p = np.zeros((nsub * P, D * F), np.float32)
    np4 = np.zeros((nsup * P, F), np.float32)
    idxw = np.zeros((nsup * P, ncol_max), np.int16)

    pvec = np.arange(P)
    for sp in range(nsup):
        k = int(k_sup[sp])
        if k == 0:
            continue
        # [GSUB, P] node ids for the 4 subtiles
        v0s = subs[sp * GSUB : (sp + 1) * GSUB]
        nodes = node_order[v0s[:, None] + pvec[None, :]]  # [4, 128]
        kvn = kv[nodes]  # [4, 128]
        rr = np.arange(k)
        real = rr[None, None, :] < kvn[:, :, None]  # [4, 128, k]
        dsel = dord[nodes][:, :, :k]  # [4, 128, k] direction of rank r
        # packed ini: [4, 128, k, F]
        ini_sel = ini_core[nodes[:, :, None], dsel]  # [4,128,k,F]
        ini_sel = np.where(real[..., None], ini_sel, 0.0).astype(np.float32)
        for s in range(GSUB):
            gi = sp * GSUB + s
            ini_p[gi * P : (gi + 1) * P, 0 : k * F] = ini_sel[s].reshape(P, k * F)
        # pad-count correction: row 32s+g, col u = k - kv(node 32g+u of sub s)
        npads = (k - kvn).astype(np.float32)  # [4, 128] = [s, 32g+u]
        for s in range(GSUB):
            np4[sp * P + 32 * s : sp * P + 32 * s + 4, :] = npads[s].reshape(4, 32)
        # gather indices: flat i = c*128 + p, c = s*k + r
        idx = adj_core[nodes[:, :, None], dsel]  # [4, 128, k]
        idx = np.where(real, idx, V)  # pads gather the zero row
        idx = _remap_rows(idx)  # chunk-major table layout
        flat = idx.transpose(0, 2, 1).reshape(GSUB * k * P)  # [(s,r,p)]
        wrapped = flat.reshape(-1, 16).T  # [16, ncol]
        ncol = GSUB * k * P // 16
        idxw[sp * P : (sp + 1) * P, 0:ncol] = np.tile(wrapped, (8, 1))
    return ini_p, np4, idxw


def _unpack_core(out_dev, node_order, dord, kv, k_sup, out_full_core, vsh=VSH):
    """Block-layout packed output -> [vsh, D, F] (zeros pre-filled)."""
    subs = _sub_starts(vsh)
    nsub = len(subs)
    pvec = np.arange(P)
    for i, v0 in enumerate(subs):
        k = int(k_sup[i // GSUB])
        if k == 0:
            continue
        blk = out_dev[i * P : (i + 1) * P, 0 : k * F]
        # rows (a,o) cols (r,u): value(node 32a+u, rank r, feat o)
        blk = blk.reshape(4, 32, k, 32)  # [a, o, r, u]
        unb = blk.transpose(0, 3, 2, 1).reshape(P, k, F)  # [node, r, o]
        nodes = node_order[v0 + pvec]  # [128]
        kvn = kv[nodes]
        rr = np.arange(k)
        real = rr[None, :] < kvn[:, None]  # [128, k]
        nidx = np.repeat(nodes, k)[real.ravel()]
        didx = dord[nodes][:, :k].ravel()[real.ravel()]
        out_full_core[nidx, didx] = unb.reshape(P * k, F)[real.ravel()]


_NC_CACHE = {}


def _run(
    inputs,
    initial_states,
    mask,
    W_kernel,
    W_bias,
    a_kernel,
    adj_lst,
    mask_index,
    trace=False,
):
    from concourse.bass_utils import run_bass_kernel_spmd

    inputs = np.asarray(inputs, np.float32)
    initial_states = np.asarray(initial_states, np.float32)
    mask = np.asarray(mask, np.float32)
    adj = np.asarray(adj_lst)
    # pad ids (== mask_index) gather the zeroed pad row at V
    adj = np.where(adj == np.asarray(mask_index), V, adj).astype(np.int32)
    w4q, a4q, o4rep, bblk = _consts(
        np.asarray(W_kernel, np.float32),
        np.asarray(W_bias, np.float32),
        np.asarray(a_kernel, np.float32),
    )

    # per-core packing
    packs = []
    subs = np.array(_sub_starts(VSH))
    nsub = len(subs)
    nsup = nsub // GSUB
    k_sub_all = np.zeros((NCORES, nsub), np.int64)
    for c in range(NCORES):
        b, h = c // 2, c % 2
        sl = slice(h * VSH, (h + 1) * VSH)
        node_order, dord, kv, k_sub = _pack_core(adj[b, sl], mask[b, sl], None)
        packs.append((node_order, dord, kv))
        k_sub_all[c] = k_sub
    k_sched = k_sub_all.max(axis=0)  # same program on every core
    k_sup = np.maximum.reduce(
        [k_sched[i::GSUB][:nsup] for i in range(GSUB)]
    )

    key = tuple(int(x) for x in k_sup)
    if key not in _NC_CACHE:
        _NC_CACHE.clear()
        _NC_CACHE[key] = build_nc(k_sup)
    nc = _NC_CACHE[key]

    in_maps = []
    for c in range(NCORES):
        b, h = c // 2, c % 2
        sl = slice(h * VSH, (h + 1) * VSH)
        node_order, dord, kv = packs[c]
        ini_p, np4, idxw = _pack_inputs(
            adj[b, sl], initial_states[b, sl], node_order, dord, kv, k_sup
        )
        in_maps.append(
            {
                "x": np.ascontiguousarray(inputs[b, sl]),
                "ini": ini_p,
                "np4": np4,
                "idxw": idxw,
                "w4q": w4q,
                "a4q": a4q,
                "o4rep": o4rep,
                "bblk": bblk,
            }
        )

    res = run_bass_kernel_spmd(
        nc, in_maps, list(range(NCORES)),
        trace=trace, trace_cores=[0] if trace else None,
    )
    out = np.zeros((B, V, D, OUT), np.float32)
    for c in range(NCORES):
        b, h = c // 2, c % 2
        node_order, dord, kv = packs[c]
        _unpack_core(
            res.results[c]["out"], node_order, dord, kv, k_sup,
            out[b, h * VSH : (h + 1) * VSH],
        )
    return out, res


def kernel(
    inputs,
    initial_states,
    mask,
    W_kernel,
    W_bias,
    a_kernel,
    adj_lst,
    mask_index,
):
    out, _ = _run(
        inputs, initial_states, mask, W_kernel, W_bias, a_kernel,
        adj_lst, mask_index,
    )
    return out
